# revision 14
# baseline (speedup 1.0000x reference)
"""GAT (2-layer, PyG-style) on 8 Trainium2 NeuronCores via Bass/Tile.

Strategy (dst-major graph-parallel):
  - Nodes are partitioned across 8 cores by dst id (6250 each). Edges live on
    the core owning their destination.
  - Per core, own dsts are degree-sorted and grouped into 49 blocks of 128.
    Each block is a [128 dst-partitions x S slots] grid; slot (d, s) holds the
    s-th incoming edge of block-dst d.  Per-edge work is then pure free-dim
    DVE work; segment-softmax and aggregation are free-dim reductions.
  - Per-edge source features+attention are fetched with dma_gather from a
    DRAM table computed on-device (phase A: h = x @ W1ext).  Gather indices
    are int16, so the table is split at row 32768 (lo/hi sub-grids).
  - adst (attention of the block's own dsts) is recomputed per block from a
    host-permuted copy of x (avoids any by-dst gather).
  - Layer 1 output (elu'd) returns to host, which reassembles/transposes and
    launches layer 2 (same machinery, 1 head, 16 classes).

kernel(**inputs) takes FULL unsharded inputs, returns the FULL [50000, 16]
output.  Host-side numpy does sharding/index prep only; all model math runs
on the NeuronCores.
"""

import os
import sys

import numpy as np

sys.path.insert(0, "/opt/trn_rl_repo")

import concourse.bacc as bacc
import concourse.bass as bass
import concourse.mybir as mybir
import concourse.tile as tile
from concourse.bass_utils import run_bass_kernel_spmd

F32 = mybir.dt.float32
BF16 = mybir.dt.bfloat16
I16 = mybir.dt.int16

N = 50000
NC = 8
OWN = N // NC            # 6250
FIN = 128
HID = 16
HEADS = 8
FH1 = HEADS * HID        # 128
CLS = 16
NEG = 0.2
HALF = 32768
NPAD = 50048             # 391 * 128
NCHUNK = NPAD // 128     # 391
BLKS = 49                # ceil(6250/128)
OWNPAD = BLKS * 128      # 6272

# Layer table layouts (f32-typed rows; gather moves bytes).
# L1 row (128 f32 = 512B): [h bf16 x128 (f32 cols 0:64) | psum-junk | asrc
# f32 x8 at cols 120:128].  Filled by ACT cast copy (h) + one DVE copy of
# psum cols 72:136 -> st cols 64:128, so every byte is initialized.
ROW1 = 128
A1OFF = 120
# L2 row (64 f32 = 256B): [h2 bf16 x16 (f32 cols 0:8) | zeros | asrc2 at 63]
ROW2 = 64
A2OFF = 63


# ---------------------------------------------------------------- host prep

def _prep(edge_index):
    """Build per-core grid structures from the edge list. Pure numpy."""
    ei = np.asarray(edge_index)
    loop = np.arange(N, dtype=np.int64)
    src = np.concatenate([ei[0].astype(np.int64), loop])
    dst = np.concatenate([ei[1].astype(np.int64), loop])

    cores = []
    # per-core, per-block max lo/hi degree -> uniform grids
    lodeg_all = np.zeros((NC, OWNPAD), np.int64)
    hideg_all = np.zeros((NC, OWNPAD), np.int64)
    order_all = []
    for c in range(NC):
        m = (dst >= c * OWN) & (dst < (c + 1) * OWN)
        s_c = src[m]
        d_c = dst[m] - c * OWN
        deg = np.bincount(d_c, minlength=OWN)
        sigma = np.argsort(-deg, kind="stable")  # degree desc; zero-deg last
        rank = np.empty(OWN, np.int64)
        rank[sigma] = np.arange(OWN)
        lo = s_c < HALF
        lodeg = np.bincount(d_c[lo], minlength=OWN)
        hideg = deg - lodeg
        lodeg_all[c, :OWN] = lodeg[sigma]
        hideg_all[c, :OWN] = hideg[sigma]
        order_all.append((s_c, d_c, sigma, rank))

    blk_lo = lodeg_all.reshape(NC, BLKS, 128).max(axis=2)
    blk_hi = hideg_all.reshape(NC, BLKS, 128).max(axis=2)
    S_LO = blk_lo.max(axis=0).astype(int)         # [BLKS]
    S_HI = blk_hi.max(axis=0).astype(int)
    S_LO = np.maximum(S_LO, 1)
    S_HI = np.maximum(S_HI, 1)
    LOP = np.concatenate([[0], np.cumsum(S_LO)]).astype(int)   # lo col prefix
    HIP = np.concatenate([[0], np.cumsum(S_HI)]).astype(int)
    MP = np.concatenate([[0], np.cumsum(S_LO + S_HI)]).astype(int)
    SLO, SHI = int(LOP[-1]), int(HIP[-1])
    STOT = int(MP[-1])

    for c in range(NC):
        s_c, d_c, sigma, rank = order_all[c]
        r = rank[d_c]                      # dst rank of each edge
        blk = r // 128
        p = r % 128
        lo = s_c < HALF
        # slot within (blk, p, half): order of appearance
        key = blk * (128 * 2) + p * 2 + (~lo).astype(np.int64)
        order = np.argsort(key, kind="stable")
        ks = key[order]
        # position within each (blk,p,half) group
        uniq, first_idx, counts = np.unique(ks, return_index=True,
                                            return_counts=True)
        slot = np.arange(len(ks)) - np.repeat(first_idx, counts)
        # gather index arrays, int16, position i -> (partition i%128, col i//128)
        idx_lo = np.zeros((SLO, 128), np.int16)    # [col, partition]
        idx_hi = np.zeros((SHI, 128), np.int16)
        mask = np.zeros((STOT, 128), np.float32)   # [col, partition]
        eb, ep_, es = blk[order], p[order], slot
        el = lo[order]
        esrc = s_c[order]
        col_lo = LOP[eb] + es
        col_hi = HIP[eb] + es
        i_lo = el
        idx_lo[col_lo[i_lo], ep_[i_lo]] = esrc[i_lo].astype(np.int16)
        i_hi = ~el
        idx_hi[col_hi[i_hi], ep_[i_hi]] = (esrc[i_hi] - HALF).astype(np.int16)
        mcol = np.where(el, MP[eb] + es, MP[eb] + S_LO[eb] + es)
        mask[mcol, ep_] = 1.0
        # sanity: every slot unique
        assert es.max() < max(S_LO.max(), S_HI.max()) + 1
        cores.append(dict(
            sigma=sigma,
            idx_lo=idx_lo.T.copy(),    # [128 part, SLO cols] -> wrap below
            idx_hi=idx_hi.T.copy(),
            mask=mask.T.copy(),        # [128, STOT]
        ))

    grids = dict(S_LO=S_LO, S_HI=S_HI, LOP=LOP, HIP=HIP, MP=MP,
                 SLO=SLO, SHI=SHI, STOT=STOT)
    return cores, grids


def _wrap_idx(idx_pc):
    """[128, COLS] per-(partition,col) int16 -> dma_gather idx tile layout.

    dma_gather reads idx position i at sbuf [i%16, i//16] (int16), replicated
    across all 8 groups of 16 partitions.  Position i maps to output
    (partition i%128, col i//128).
    """
    P, C = idx_pc.shape
    assert P == 128
    flat = idx_pc.T.reshape(-1)            # position i = p + 128*c
    n16 = (len(flat) + 15) // 16
    t = np.zeros((16, n16), np.int16)
    t[np.arange(len(flat)) % 16, np.arange(len(flat)) // 16] = flat
    return np.tile(t, (8, 1))              # [128, n16]


# ------------------------------------------------------------- bass builder

def _build_layer(grids, layer):
    """One GAT layer as a Bass SPMD program.

    layer 1: FIN=128 in, 8 heads x 16 -> out 128 (elu'd x2)
    layer 2: 128 in, 1 head x 16 -> out 16 (+bias only)
    """
    S_LO, S_HI = grids["S_LO"], grids["S_HI"]
    LOP, HIP, MP = grids["LOP"], grids["HIP"], grids["MP"]
    SLO, SHI, STOT = grids["SLO"], grids["SHI"], grids["STOT"]

    if layer == 1:
        FH, AH, ROW, AOFF = FH1, HEADS, ROW1, A1OFF
        WCOLS = FH + AH          # 136: [W1 | W1@Asrc]
        FOUT = FH1
        CP0, CP1 = 72, 64        # DVE copy psum[:, CP0:WCOLS] -> st[:, CP1:]
    else:
        FH, AH, ROW, AOFF = CLS, 1, ROW2, A2OFF
        WCOLS = 64               # [W2 | zeros | W2@Asrc2 at col 63]
        FOUT = CLS
        CP0, CP1 = 8, 8

    nc = bacc.Bacc("TRN2", target_bir_lowering=False, debug=False,
                   num_devices=NC)
    xt = nc.declare_dram_parameter("xt", [128, NPAD], BF16, isOutput=False)
    xpermt = nc.declare_dram_parameter("xpermt", [128, OWNPAD], BF16,
                                       isOutput=False)
    wext = nc.declare_dram_parameter("wext", [128, WCOLS], BF16,
                                     isOutput=False)
    wadst = nc.declare_dram_parameter("wadst", [128, AH], BF16,
                                      isOutput=False)
    brow = nc.declare_dram_parameter("brow", [128, FOUT], F32, isOutput=False)
    idxlo = nc.declare_dram_parameter("idxlo", [128, 8 * SLO], I16,
                                      isOutput=False)
    idxhi = nc.declare_dram_parameter("idxhi", [128, 8 * SHI], I16,
                                      isOutput=False)
    maskp = nc.declare_dram_parameter("maskp", [128, STOT], F32,
                                      isOutput=False)
    out = nc.declare_dram_parameter("out", [OWNPAD, FOUT], F32, isOutput=True)
    th = nc.dram_tensor("th", [NPAD, ROW], F32)

    with tile.TileContext(nc) as tc:
        with (
            tc.tile_pool(name="const", bufs=1) as cpool,
            tc.tile_pool(name="xa", bufs=4) as xpool,
            tc.tile_pool(name="stage", bufs=4) as spool,
            tc.tile_pool(name="psA", bufs=2, space="PSUM") as psA,
            tc.tile_pool(name="psB", bufs=2, space="PSUM") as psB,
            tc.tile_pool(name="gath", bufs=2) as gpool,
            tc.tile_pool(name="ep", bufs=2) as epool,
            tc.tile_pool(name="msg", bufs=2) as mpool,
            tc.tile_pool(name="fin", bufs=3) as fpool,
        ):
            # constants
            w_sb = cpool.tile([128, WCOLS], BF16)
            nc.sync.dma_start(w_sb[:], wext[:])
            wa_sb = cpool.tile([128, AH], BF16)
            nc.sync.dma_start(wa_sb[:], wadst[:])
            b_sb = cpool.tile([128, FOUT], F32)
            nc.sync.dma_start(b_sb[:], brow[:])
            il_sb = cpool.tile([128, 8 * SLO], I16)
            nc.sync.dma_start(il_sb[:], idxlo[:])
            ih_sb = cpool.tile([128, 8 * SHI], I16)
            nc.sync.dma_start(ih_sb[:], idxhi[:])
            mk_sb = cpool.tile([128, STOT], F32)
            nc.sync.dma_start(mk_sb[:], maskp[:])

            # ---- phase A: th[n] = [h(n) bf16 | asrc(n) f32 | junk]
            for i in range(NCHUNK):
                xt_t = xpool.tile([128, 128], BF16)
                nc.sync.dma_start(xt_t[:], xt[:, i * 128:(i + 1) * 128])
                ph = psA.tile([128, WCOLS], F32)
                nc.tensor.matmul(ph[:], xt_t[:], w_sb[:], start=True,
                                 stop=True)
                st = spool.tile([128, ROW], F32)
                # h -> bf16 (cast on copy); tail cols f32 incl asrc
                nc.scalar.copy(st.bitcast(BF16)[:, 0:FH], ph[:, 0:FH])
                nc.vector.tensor_copy(st[:, CP1:ROW], ph[:, CP0:WCOLS])
                nc.sync.dma_start(th[i * 128:(i + 1) * 128, :], st[:])

            tc.strict_bb_all_engine_barrier()

            # ---- phase B: per 128-dst block
            for j in range(BLKS):
                Sl, Sh = int(S_LO[j]), int(S_HI[j])
                S = Sl + Sh
                # adst for this block's dsts, recomputed from permuted x
                xp_t = xpool.tile([128, 128], BF16, tag="xp")
                nc.sync.dma_start(xp_t[:],
                                  xpermt[:, j * 128:(j + 1) * 128])
                pa = psB.tile([128, AH], F32)
                nc.tensor.matmul(pa[:], xp_t[:], wa_sb[:], start=True,
                                 stop=True)
                adst = fpool.tile([128, AH], F32, tag="adst")
                nc.vector.tensor_copy(adst[:], pa[:])

                g = gpool.tile([128, S, ROW], F32, tag="g")
                nc.gpsimd.dma_gather(
                    g[:, 0:Sl, :], th[0:HALF, :],
                    il_sb[:, 8 * LOP[j]: 8 * (LOP[j] + Sl)],
                    num_idxs=128 * Sl, num_idxs_reg=128 * Sl, elem_size=ROW,
                    single_packet=False)
                nc.gpsimd.dma_gather(
                    g[:, Sl:S, :], th[HALF:NPAD, :],
                    ih_sb[:, 8 * HIP[j]: 8 * (HIP[j] + Sh)],
                    num_idxs=128 * Sh, num_idxs_reg=128 * Sh, elem_size=ROW,
                    single_packet=False)

                # e = lrelu(asrc + adst); p = exp(e) * mask
                asrc = g[:, :, AOFF:AOFF + AH]          # [128, S, AH] f32
                e = epool.tile([128, S, AH], F32, tag="e")
                nc.vector.tensor_tensor(
                    e[:], asrc,
                    adst[:].unsqueeze(1).broadcast_to([128, S, AH]),
                    op=mybir.AluOpType.add)
                e2 = epool.tile([128, S, AH], F32, tag="e2")
                nc.vector.tensor_scalar_mul(e2[:], e[:], NEG)
                nc.vector.tensor_tensor(e[:], e[:], e2[:],
                                        op=mybir.AluOpType.max)
                pt = epool.tile([128, S, AH], F32, tag="p")
                nc.scalar.activation(pt[:], e[:],
                                     mybir.ActivationFunctionType.Exp)
                pm = epool.tile([128, S, AH], F32, tag="pm")
                nc.vector.tensor_tensor(
                    pm[:], pt[:],
                    mk_sb[:, MP[j]:MP[j] + S].unsqueeze(2)
                         .broadcast_to([128, S, AH]),
                    op=mybir.AluOpType.mult)

                den = fpool.tile([128, AH], F32, tag="den")
                nc.vector.tensor_reduce(den[:],
                                        pm[:].transpose([0, 2, 1]),
                                        axis=mybir.AxisListType.X,
                                        op=mybir.AluOpType.add)
                nc.vector.tensor_scalar_add(den[:], den[:], 1e-16)
                rec = fpool.tile([128, AH], F32, tag="rec")
                nc.vector.reciprocal(rec[:], den[:])
                al = epool.tile([128, S, AH], F32, tag="al")
                nc.vector.tensor_tensor(
                    al[:], pm[:],
                    rec[:].unsqueeze(1).broadcast_to([128, S, AH]),
                    op=mybir.AluOpType.mult)

                # msg = h_gath * alpha (per head); out_un = sum over slots
                hview = g.bitcast(BF16)[:, :, 0:FH]
                hview = hview.rearrange("p s (h c) -> p s h c", c=HID)
                msg = mpool.tile([128, S, AH, HID], F32, tag="msg")
                nc.vector.tensor_tensor(
                    msg[:], hview,
                    al[:].unsqueeze(3).broadcast_to([128, S, AH, HID]),
                    op=mybir.AluOpType.mult)
                outun = fpool.tile([128, FOUT], F32, tag="outun")
                mv = msg[:].rearrange("p s h c -> p (h c) s")
                nc.vector.tensor_reduce(outun[:], mv,
                                        axis=mybir.AxisListType.X,
                                        op=mybir.AluOpType.add)

                fin = fpool.tile([128, FOUT], F32, tag="fin")
                if layer == 1:
                    # x2 = elu(outun + b1)
                    nc.vector.tensor_tensor(outun[:], outun[:], b_sb[:],
                                            op=mybir.AluOpType.add)
                    mn = fpool.tile([128, FOUT], F32, tag="mn")
                    nc.vector.tensor_scalar_min(mn[:], outun[:], 0.0)
                    ex = fpool.tile([128, FOUT], F32, tag="ex")
                    nc.scalar.activation(ex[:], mn[:],
                                         mybir.ActivationFunctionType.Exp)
                    mx = fpool.tile([128, FOUT], F32, tag="mx")
                    nc.vector.tensor_scalar_max(mx[:], outun[:], 0.0)
                    nc.vector.tensor_tensor(ex[:], ex[:], mx[:],
                                            op=mybir.AluOpType.add)
                    nc.vector.tensor_scalar_add(fin[:], ex[:], -1.0)
                else:
                    nc.vector.tensor_tensor(fin[:], outun[:], b_sb[:],
                                            op=mybir.AluOpType.add)
                nc.sync.dma_start(out[j * 128:(j + 1) * 128, :], fin[:])

    nc.compile()
    return nc


# --------------------------------------------------------------- execution

_CACHE = {}
TRACE = os.environ.get("GAT_TRACE", "0") == "1"
RUN_KW = {}


def _to_bf16(a):
    return np.asarray(a, np.float32).astype(mybir.dt.np(BF16))


def _amat(att, fh, hid, heads):
    """[heads, hid] attention vec -> [fh, heads] block-diag matrix."""
    m = np.zeros((fh, heads), np.float32)
    for h in range(heads):
        m[h * hid:(h + 1) * hid, h] = att[h]
    return m


def kernel(x, edge_index, W1, att_src1, att_dst1, b1, W2, att_src2, att_dst2,
           b2):
    x = np.asarray(x, np.float32)
    ei = np.asarray(edge_index)
    key = "prep"
    if key not in _CACHE:
        _CACHE[key] = _prep(ei)
    cores, grids = _CACHE[key]

    if "nc1" not in _CACHE:
        _CACHE["nc1"] = _build_layer(grids, 1)
        _CACHE["nc2"] = _build_layer(grids, 2)
    nc1, nc2 = _CACHE["nc1"], _CACHE["nc2"]

    # ---- layer 1 inputs
    W1 = np.asarray(W1, np.float32)
    As1 = _amat(np.asarray(att_src1, np.float32), FH1, HID, HEADS)
    Ad1 = _amat(np.asarray(att_dst1, np.float32), FH1, HID, HEADS)
    w1ext = _to_bf16(np.concatenate([W1, W1 @ As1], axis=1))     # [128,136]
    w1adst = _to_bf16(W1 @ Ad1)                                  # [128,8]
    b1row = np.tile(np.asarray(b1, np.float32)[None, :], (128, 1))

    xpad = np.zeros((NPAD, FIN), np.float32)
    xpad[:N] = x
    xt = _to_bf16(xpad.T.copy())                                 # [128,NPAD]

    in_maps = []
    for c in range(NC):
        sig = cores[c]["sigma"]
        xperm = np.zeros((OWNPAD, FIN), np.float32)
        xperm[:OWN] = x[c * OWN + sig]
        in_maps.append(dict(
            xt=xt, wext=w1ext, wadst=w1adst, brow=b1row,
            xpermt=_to_bf16(xperm.T.copy()),
            idxlo=_wrap_idx(cores[c]["idx_lo"]),
            idxhi=_wrap_idx(cores[c]["idx_hi"]),
            maskp=cores[c]["mask"],
        ))
    res1 = run_bass_kernel_spmd(nc1, in_maps, list(range(NC)),
                                trace=TRACE, **RUN_KW)

    x2 = np.zeros((N, FH1), np.float32)
    for c in range(NC):
        sig = cores[c]["sigma"]
        x2[c * OWN + sig] = res1.results[c]["out"][:OWN]

    # ---- layer 2 inputs
    W2 = np.asarray(W2, np.float32)
    As2 = _amat(np.asarray(att_src2, np.float32), CLS, CLS, 1)
    Ad2 = _amat(np.asarray(att_dst2, np.float32), CLS, CLS, 1)
    w2ext = _to_bf16(np.concatenate(
        [W2, np.zeros((FH1, 64 - CLS - 1), np.float32), W2 @ As2],
        axis=1))                                                 # [128,64]
    w2adst = _to_bf16(W2 @ Ad2)                                  # [128,1]
    b2row = np.tile(np.asarray(b2, np.float32)[None, :], (128, 1))

    x2pad = np.zeros((NPAD, FH1), np.float32)
    x2pad[:N] = x2
    x2t = _to_bf16(x2pad.T.copy())

    in_maps2 = []
    for c in range(NC):
        sig = cores[c]["sigma"]
        xperm = np.zeros((OWNPAD, FH1), np.float32)
        xperm[:OWN] = x2[c * OWN + sig]
        in_maps2.append(dict(
            xt=x2t, wext=w2ext, wadst=w2adst, brow=b2row,
            xpermt=_to_bf16(xperm.T.copy()),
            idxlo=in_maps[c]["idxlo"],
            idxhi=in_maps[c]["idxhi"],
            maskp=in_maps[c]["maskp"],
        ))
    res2 = run_bass_kernel_spmd(nc2, in_maps2, list(range(NC)),
                                trace=TRACE, **RUN_KW)

    outf = np.zeros((N, CLS), np.float32)
    for c in range(NC):
        sig = cores[c]["sigma"]
        outf[c * OWN + sig] = res2.results[c]["out"][:OWN]
    kernel.last_results = (res1, res2)
    return outf
